# revision 7
# baseline (speedup 1.0000x reference)
"""TRN2 Bass kernel for margin-based triplet loss (nn_Criterion_28278064676994).

Sharding: triplets data-parallel across 8 NeuronCores (62500 each, padded to
65536 = 16 chunks x 4096). The batch embedding table is NOT gathered on the
host: each core keeps the full bf16 table in HBM and performs the row gather
on-device with gpsimd.dma_gather (SWDGE indirect DMA), 12288 rows per chunk
(anchor/positive/negative interleaved in one index list).

Per chunk (gathered tile g = [128 part, 96 slots, 128 dims] bf16):
    d1 = a - p ; d2 = a - n          (DVE, in-place over the p/n slots)
    s1 = d1^2 ; s2 = d2^2            (ACT, in-place)
    dsq = reduce_add over dims       (DVE) -> dsq_ap/dsq_an [128, 512] f32
Fused tail over [128, 512]:
    d_ap = sqrt(dsq + eps), pos = relu(d_ap - b + M), neg = relu(b + M - d_an)
    s = (pos + neg) * mask, ind = s > 0, partials = [sum(s), sum(ind)]
Partials are all-reduced over partitions (gpsimd) and the 8 per-core partial
pairs are combined on host: loss = where(cnt==0, tot, tot/max(cnt,1)).
"""
import numpy as np
from contextlib import ExitStack

MARGIN = 0.2
EPS = 1e-8
NCORES = 8
D = 128
N_ROWS = 16384      # batch rows
TC = 62500          # triplets per core
F = 32              # free slots per chunk per partition
CHUNK = 128 * F     # 4096 triplets per chunk
NCH = 16            # chunks per core
TPAD = NCH * CHUNK  # 65536
GIDX = 3 * CHUNK    # gathered rows per chunk (a,p,n)
IDXCOLS = GIDX // 16

_CACHE = {}


def _build_nc():
    if "nc" in _CACHE:
        return _CACHE["nc"]

    import concourse.bacc as bacc
    import concourse.tile as tile
    import concourse.mybir as mybir

    # 64 KiB/partition SWDGE carveout -> 1024-descriptor rings, so several
    # 4096-row gathers (257 descs/engine each) can be in flight at once.
    nc = bacc.Bacc("TRN2", target_bir_lowering=False, debug=False,
                   num_devices=NCORES, dynamic_dma_scratch_size=65536)
    f32 = mybir.dt.float32
    bf16 = mybir.dt.bfloat16
    i16 = mybir.dt.int16
    bt_d = nc.dram_tensor("bt", (N_ROWS, D), bf16, kind="ExternalInput").ap()
    ix_d = nc.dram_tensor("ix", (NCH, 128, IDXCOLS), i16,
                          kind="ExternalInput").ap()
    bv_d = nc.dram_tensor("bv", (128, NCH * F), f32, kind="ExternalInput").ap()
    mk_d = nc.dram_tensor("mk", (128, NCH * F), f32, kind="ExternalInput").ap()
    out_d = nc.dram_tensor("out", (1, 2), f32, kind="ExternalOutput").ap()

    A = mybir.AluOpType
    with tile.TileContext(nc) as tc, ExitStack() as ctx:
        sb = ctx.enter_context(tc.tile_pool(name="sb", bufs=2))
        per = ctx.enter_context(tc.tile_pool(name="per", bufs=1))

        dsq_ap = per.tile([128, NCH * F], f32)
        dsq_an = per.tile([128, NCH * F], f32)
        b_t = per.tile([128, NCH * F], f32)
        mk_t = per.tile([128, NCH * F], f32)
        nc.sync.dma_start(b_t[:], bv_d[:])
        nc.sync.dma_start(mk_t[:], mk_d[:])

        for c in range(NCH):
            ix = sb.tile([128, IDXCOLS], i16, tag="ix")
            nc.sync.dma_start(ix[:], ix_d[c])
            g = sb.tile([128, 3 * F, D], bf16, tag="g")
            # three 4096-row gathers (a/p/n); a single 12288-row gather would
            # need 769 descs/engine in one instruction and overflow the ring
            for t in range(3):
                nc.gpsimd.dma_gather(
                    g[:, t * F:(t + 1) * F, :], bt_d[:],
                    ix[:, t * (CHUNK // 16):(t + 1) * (CHUNK // 16)],
                    CHUNK, CHUNK, D, single_packet=False)
            # d1 = a - p (over p's slots), d2 = a - n (over n's slots)
            nc.vector.tensor_tensor(out=g[:, F:2 * F, :], in0=g[:, 0:F, :],
                                    in1=g[:, F:2 * F, :], op=A.subtract)
            nc.vector.tensor_tensor(out=g[:, 2 * F:3 * F, :], in0=g[:, 0:F, :],
                                    in1=g[:, 2 * F:3 * F, :], op=A.subtract)
            nc.scalar.activation(g[:, F:2 * F, :], g[:, F:2 * F, :],
                                 mybir.ActivationFunctionType.Square)
            nc.scalar.activation(g[:, 2 * F:3 * F, :], g[:, 2 * F:3 * F, :],
                                 mybir.ActivationFunctionType.Square)
            nc.vector.tensor_reduce(
                dsq_ap[:, c * F:(c + 1) * F], g[:, F:2 * F, :],
                axis=mybir.AxisListType.X, op=A.add)
            nc.vector.tensor_reduce(
                dsq_an[:, c * F:(c + 1) * F], g[:, 2 * F:3 * F, :],
                axis=mybir.AxisListType.X, op=A.add)

        # tail over [128, NCH*F]
        dap = per.tile([128, NCH * F], f32)
        dan = per.tile([128, NCH * F], f32)
        epsb = per.tile([128, 1], f32)
        nc.vector.memset(epsb[:], EPS)
        nc.scalar.activation(dap[:], dsq_ap[:],
                             mybir.ActivationFunctionType.Sqrt, bias=epsb[:])
        nc.scalar.activation(dan[:], dsq_an[:],
                             mybir.ActivationFunctionType.Sqrt, bias=epsb[:])
        pos = per.tile([128, NCH * F], f32)
        neg = per.tile([128, NCH * F], f32)
        # pos = (dap + M) - b ; neg = (b + M) - dan
        nc.vector.scalar_tensor_tensor(
            out=pos[:], in0=dap[:], scalar=MARGIN, in1=b_t[:],
            op0=A.add, op1=A.subtract)
        nc.vector.scalar_tensor_tensor(
            out=neg[:], in0=b_t[:], scalar=MARGIN, in1=dan[:],
            op0=A.add, op1=A.subtract)
        nc.vector.tensor_scalar_max(out=pos[:], in0=pos[:], scalar1=0.0)
        nc.vector.tensor_scalar_max(out=neg[:], in0=neg[:], scalar1=0.0)
        s_t = per.tile([128, NCH * F], f32)
        nc.vector.tensor_tensor(out=s_t[:], in0=pos[:], in1=neg[:], op=A.add)
        nc.vector.tensor_tensor(out=s_t[:], in0=s_t[:], in1=mk_t[:], op=A.mult)
        ind = per.tile([128, NCH * F], f32)
        nc.vector.tensor_scalar(out=ind[:], in0=s_t[:], scalar1=0.0,
                                scalar2=None, op0=A.is_gt)
        pr = per.tile([128, 2], f32)
        nc.vector.tensor_reduce(pr[:, 0:1], s_t[:],
                                axis=mybir.AxisListType.X, op=A.add)
        nc.vector.tensor_reduce(pr[:, 1:2], ind[:],
                                axis=mybir.AxisListType.X, op=A.add)
        import concourse.bass_isa as bass_isa
        red = per.tile([128, 2], f32)
        nc.gpsimd.partition_all_reduce(red[:], pr[:], channels=128,
                                       reduce_op=bass_isa.ReduceOp.add)
        nc.sync.dma_start(out_d[:], red[0:1, :])

    nc.compile()
    _CACHE["nc"] = nc
    return nc


def _build_runner():
    if "runner" in _CACHE:
        return _CACHE["runner"]
    nc = _build_nc()
    _CACHE["runner"] = (nc, _make_runner_factory(nc))
    return _CACHE["runner"]


def _make_runner_factory(nc):
    """Returns runner(in_maps) -> run_fn, mirroring bass2jax.run_bass_via_pjrt
    but with a reusable jitted callable (inputs staged on device once)."""
    import jax
    import numpy as _np
    from jax.sharding import Mesh, PartitionSpec
    from jax.experimental.shard_map import shard_map
    import concourse.mybir as mybir
    from concourse.bass2jax import (
        _bass_exec_p, install_neuronx_cc_hook, partition_id_tensor)

    install_neuronx_cc_hook()
    partition_name = nc.partition_id_tensor.name if nc.partition_id_tensor else None
    in_names, out_names, out_avals, zero_outs = [], [], [], []
    for alloc in nc.m.functions[0].allocations:
        if not isinstance(alloc, mybir.MemoryLocationSet):
            continue
        name = alloc.memorylocations[0].name
        if alloc.kind == "ExternalInput":
            if name != partition_name:
                in_names.append(name)
        elif alloc.kind == "ExternalOutput":
            out_names.append(name)
            shape = tuple(alloc.tensor_shape)
            dtype = mybir.dt.np(alloc.dtype)
            out_avals.append(jax.core.ShapedArray(shape, dtype))
            zero_outs.append(_np.zeros(shape, dtype))
    n_params, n_outs = len(in_names), len(out_avals)
    all_in = list(in_names) + list(out_names)
    if partition_name is not None:
        all_in.append(partition_name)

    def _body(*args):
        operands = list(args)
        if partition_name is not None:
            operands.append(partition_id_tensor())
        return tuple(_bass_exec_p.bind(
            *operands, out_avals=tuple(out_avals), in_names=tuple(all_in),
            out_names=tuple(out_names), lowering_input_output_aliases=(),
            sim_require_finite=True, sim_require_nnan=True, nc=nc))

    devices = jax.devices()[:NCORES]
    mesh = Mesh(_np.asarray(devices), ("core",))
    sharded = jax.jit(
        shard_map(_body, mesh=mesh,
                  in_specs=(PartitionSpec("core"),) * (n_params + n_outs),
                  out_specs=(PartitionSpec("core"),) * n_outs,
                  check_rep=False),
        keep_unused=True)
    sharding = jax.sharding.NamedSharding(mesh, PartitionSpec("core"))

    def runner(in_maps):
        concat_in = [
            _np.concatenate([_np.asarray(in_maps[c][nm]) for c in range(NCORES)],
                            axis=0)
            for nm in in_names
        ]
        dev_in = [jax.device_put(x, sharding) for x in concat_in]
        dev_zero = [
            jax.device_put(
                _np.zeros((NCORES * z.shape[0], *z.shape[1:]), z.dtype), sharding)
            for z in zero_outs
        ]

        def run_fn():
            outs = sharded(*dev_in, *dev_zero)
            jax.block_until_ready(outs)
            return [
                {nm: _np.asarray(outs[i]).reshape(NCORES, *out_avals[i].shape)[c]
                 for i, nm in enumerate(out_names)}
                for c in range(NCORES)
            ]

        return run_fn
    return runner


def _prep_in_maps(batch, beta, labels, triplets):
    """Build the 8 per-core input maps. Only index/metadata prep happens on
    the host; the embedding-row gather itself runs on-device."""
    import ml_dtypes
    batch_bf = np.ascontiguousarray(batch.astype(ml_dtypes.bfloat16))
    in_maps = []
    for c in range(NCORES):
        tr = triplets[c * TC:(c + 1) * TC]
        apn = np.zeros((TPAD, 3), np.int16)
        apn[:TC] = tr.astype(np.int16)
        # per-chunk index lists [a(4096) | p(4096) | n(4096)], one gather per
        # list; within a gather, slot i lands at dst[i % 128, i // 128, :] and
        # index i lives at partition i%16, column i//16 of its own idx slice;
        # replicate across the 8 gpsimd core groups
        ids = apn.reshape(NCH, CHUNK, 3).transpose(0, 2, 1)  # [NCH, 3, CHUNK]
        idx_t = ids.reshape(NCH, 3, CHUNK // 16, 16).transpose(0, 3, 1, 2)
        idx_t = idx_t.reshape(NCH, 16, IDXCOLS)
        idx_full = np.ascontiguousarray(np.tile(idx_t, (1, 8, 1)))
        # per-anchor beta and validity mask, laid out [part j%128, c*F + j//128]
        bvals = beta[labels[apn[:, 0].astype(np.int32)]].astype(np.float32)
        bv = bvals.reshape(NCH, F, 128).transpose(2, 0, 1).reshape(128, NCH * F)
        valid = np.zeros(TPAD, np.float32)
        valid[:TC] = 1.0
        mk = valid.reshape(NCH, F, 128).transpose(2, 0, 1).reshape(128, NCH * F)
        in_maps.append({
            "bt": batch_bf,
            "ix": idx_full,
            "bv": np.ascontiguousarray(bv),
            "mk": np.ascontiguousarray(mk),
        })
    return in_maps


def _combine(res):
    tot = sum(float(r["out"][0, 0]) for r in res)
    cnt = sum(float(r["out"][0, 1]) for r in res)
    loss = tot if cnt == 0.0 else tot / max(cnt, 1.0)
    return np.float32(loss)


def kernel(batch, beta, labels, triplets):
    batch = np.asarray(batch, dtype=np.float32)
    beta = np.asarray(beta, dtype=np.float32)
    labels = np.asarray(labels).astype(np.int64)
    triplets = np.asarray(triplets).astype(np.int64)
    T = triplets.shape[0]
    assert T == NCORES * TC, (T, NCORES * TC)

    in_maps = _prep_in_maps(batch, beta, labels, triplets)
    nc, runner_factory = _build_runner()
    run_fn = runner_factory(in_maps)
    return _combine(run_fn())


if __name__ == "__main__":
    # smoke test with random data
    rng = np.random.default_rng(0)
    batch = rng.standard_normal((16384, 128)).astype(np.float32)
    beta = np.full((1000,), 1.2, np.float32)
    labels = rng.integers(0, 1000, 16384)
    triplets = rng.integers(0, 16384, (500000, 3))
    out = kernel(batch=batch, beta=beta, labels=labels, triplets=triplets)
    print("loss:", out)
